# revision 1
# baseline (speedup 1.0000x reference)
"""Causal self-attention (B=2, L=2048, D=1024, H=16) on 8 trn2 NeuronCores.

Sharding: core c = 4*b + g handles batch b and head group g (4 heads).
Per core: QKV projection for its heads' weight columns (tensor-parallel),
flash-style causal attention for its 4 heads, and a partial output
projection over its 256 head-dims (row-parallel).  The host sums the 4
partial projections per batch and adds bproj.

Device layout: activations kept transposed (feature-major) throughout:
  xT [D, L] (f32r) -> Q^T packed per head-pair [128, L] (bf16);
  K^T stored as 4 zero-padded [128, L] bf16 tensors (head h occupies its
  64 partition rows, the other 64 rows are zero) so score matmuls run at
  full K=128 contraction with no tile_position tricks;
  V in natural [L, 4*65] bf16 layout -- col 64 of each head group is 1.0
  (produced by zero-cols in the interleaved weight + the bias ones-row
  matmul) so the attention row-sum Z rides along in the AV matmul;
  S^T tiles [k, q] so softmax needs no transposes; exp on ACT with the
  1/sqrt(hd) scale folded in; causal diagonal handled by adding a
  precomputed triangular -1e30 mask to the S^T psum.
QKV/proj matmuls run in float32r (rounded fp32); attention matmuls in
bf16 (separate overlapped weight loads, full PE rate).
"""

import sys
import types

import numpy as np


def _install_ntff_shim():
    """The container's antenv stub lacks axon_hooks; recreate it so
    run_bass_kernel_spmd(trace=True) can reach the NTFF profiler."""
    if "antenv.axon_hooks" in sys.modules:
        return
    try:
        import antenv
        from trn_agent_boot.trn_boot import _ntff_profile_via_ctypes
    except Exception:
        return
    mod = types.ModuleType("antenv.axon_hooks")
    hook = _ntff_profile_via_ctypes("/opt/axon/libaxon_pjrt.so")
    mod.get_axon_ntff_profile_hook = lambda: hook
    mod.set_axon_ntff_profile_hook = lambda h: None
    sys.modules["antenv.axon_hooks"] = mod
    antenv.axon_hooks = mod


_install_ntff_shim()

import ml_dtypes  # noqa: E402

import concourse.bass as bass  # noqa: E402
import concourse.mybir as mybir  # noqa: E402
import concourse.tile as tile  # noqa: E402
from concourse.bass_utils import run_bass_kernel_spmd  # noqa: E402
from concourse.vector_clock import ScopedClock, VectorClock  # noqa: E402

B, L, D, H = 2, 2048, 1024, 16
HD = D // H  # 64
N_CORES = 8
HPC = 4  # heads per core
CD = HPC * HD  # 256 head-dims per core
VW = HPC * (HD + 1)  # 260 interleaved V columns (64 vals + ones col per head)
SCALE = HD**-0.5  # 0.125
F32 = mybir.dt.float32
R32 = mybir.dt.float32r  # rounded-fp32 matmul format (4 bytes, np.float32)
B16 = mybir.dt.bfloat16
NPB16 = ml_dtypes.bfloat16
NEG = -1.0e30

KT = L // 128  # 16 k-tiles of 128 keys
NS = L // 512  # 4 query chunks of 512
N_DK = D // 128  # 8 feature k-tiles
AV_DELAY = 6  # AV matmul issues this many (k,h)-steps behind its exp


class _TileContext(tile.TileContext):
    """Split exit-drain sem waits to 1 per drain; this walrus build's
    CTRL codegen rejects drains with 2+ sync waits."""

    def _drain_and_barrier(self, tick_clock, wait_clock):
        g = tick_clock.global_clock
        n = len(g)
        procs = [i for i in range(n) if g[i] > 0]
        for p in procs:
            vec = [g[i] if i == p else 0 for i in range(n)]
            d = self.nc.sync.drain()
            wait_clock.add_sem_waits(d.ins, ScopedClock({None: VectorClock(vec)}))
        self.nc.all_engine_barrier()
        popped = self.nc._tile_sem_poison_stack.pop()
        assert popped is self._sem_poison
        self.nc.clear_and_free_semaphores(list(self.sems.allocated().values()))
        self.nc.all_engine_barrier()


def _split_multi_waits(nc):
    """This walrus build's codegen accepts only ONE sync wait per
    instruction; hoist extra waits onto preceding same-engine NOPs."""
    for f in nc.m.functions:
        for blk in f.blocks:
            orig = list(blk.instructions)
            expanded = []
            changed = False
            for ins in orig:
                si = ins.sync_info
                if si is not None and si.on_wait is not None and len(si.on_wait) > 1:
                    changed = True
                    waits = list(si.on_wait)
                    eng = nc.engines[ins.engine]
                    for w in waits[:-1]:
                        nop = eng.nop(nofuse=True).ins
                        # eng.nop() auto-appends to the CURRENT bb; pull it
                        # out -- we re-insert it before `ins` in ins's bb.
                        nc.cur_bb.bb.instructions.remove(nop)
                        nop.sync_info = mybir.SyncInfo(on_wait=[w], on_update=[])
                        expanded.append(nop)
                    ins.sync_info = mybir.SyncInfo(
                        on_wait=[waits[-1]], on_update=list(si.on_update or [])
                    )
                expanded.append(ins)
            if changed:
                il = blk.instructions
                for ins in list(il):
                    il.remove(ins)
                for ins in expanded:
                    il.append(ins)


def _build_program():
    nc = bass.Bass()
    xT_d = nc.dram_tensor("xT", [D, L], R32, kind="ExternalInput").ap()
    wqkv_d = nc.dram_tensor("wqkv", [D, 2 * CD + VW], R32, kind="ExternalInput").ap()
    bqk_d = nc.dram_tensor("bqk", [128, 4], F32, kind="ExternalInput").ap()
    bv_d = nc.dram_tensor("bv", [1, VW], R32, kind="ExternalInput").ap()
    wproj_d = nc.dram_tensor("wproj", [CD, D], R32, kind="ExternalInput").ap()
    bproj_d = nc.dram_tensor("bproj", [128, N_DK], F32, kind="ExternalInput").ap()
    onesr_d = nc.dram_tensor("onesr", [1, 512], R32, kind="ExternalInput").ap()
    tri_d = nc.dram_tensor("trimask", [128, 128], F32, kind="ExternalInput").ap()
    zer_d = nc.dram_tensor("zer", [64, L], B16, kind="ExternalInput").ap()
    yT_d = nc.dram_tensor("yT", [D, L], F32, kind="ExternalOutput").ap()

    mm = nc.tensor.matmul

    with _TileContext(nc) as tc, tc.tile_pool(name="sb", bufs=1) as sb, tc.tile_pool(
        name="ps", bufs=1, space="PSUM"
    ) as ps:
        # ---- constants (host-supplied; memset/affine_select of f32r
        # fail this walrus build's ISA checks) ----
        ones = sb.tile([1, 512], R32, tag="ones", bufs=1)
        nc.sync.dma_start(out=ones[:], in_=onesr_d[:])
        tri = sb.tile([128, 128], F32, tag="tri", bufs=1)
        nc.sync.dma_start(out=tri[:], in_=tri_d[:])
        bqk = sb.tile([128, 4], F32, tag="bqk", bufs=1)
        nc.sync.dma_start(out=bqk[:], in_=bqk_d[:])
        bv = sb.tile([1, VW], R32, tag="bv", bufs=1)
        nc.sync.dma_start(out=bv[:], in_=bv_d[:])
        bproj = sb.tile([128, N_DK], F32, tag="bproj", bufs=1)
        nc.sync.dma_start(out=bproj[:], in_=bproj_d[:])

        # ---- persistent SBUF tensors ----
        # xT loaded as [k, s] chunk tiles and wqkv split by column group so
        # the first QK accumulation only waits on ~2.5MB of DMA, not 11MB.
        wqkv = [
            sb.tile([128, 2 * CD + VW], R32, tag=f"wqkv{k}", bufs=1, name=f"wqkv{k}")
            for k in range(N_DK)
        ]
        wq_groups = [(0, 128), (128, 256), (256, 384), (384, 512), (512, 2 * CD + VW)]
        for k in range(N_DK):
            lo_, hi_ = wq_groups[0]
            nc.sync.dma_start(
                out=wqkv[k][:, lo_:hi_],
                in_=wqkv_d[128 * k : 128 * (k + 1), lo_:hi_],
            )
        xTc = [
            [
                sb.tile([128, 512], R32, tag=f"xT{k}_{s}", bufs=1, name=f"xT{k}_{s}")
                for s in range(NS)
            ]
            for k in range(N_DK)
        ]
        for k in range(N_DK):
            nc.sync.dma_start(
                out=xTc[k][0][:], in_=xT_d[128 * k : 128 * (k + 1), 0:512]
            )
        for lo_, hi_ in wq_groups[1:]:
            for k in range(N_DK):
                nc.sync.dma_start(
                    out=wqkv[k][:, lo_:hi_],
                    in_=wqkv_d[128 * k : 128 * (k + 1), lo_:hi_],
                )
        for s in range(1, NS):
            for k in range(N_DK):
                nc.sync.dma_start(
                    out=xTc[k][s][:],
                    in_=xT_d[128 * k : 128 * (k + 1), 512 * s : 512 * (s + 1)],
                )
        wproj = []
        for kt in range(2):
            t = sb.tile([128, D], R32, tag=f"wproj{kt}", bufs=1)
            nc.sync.dma_start(out=t[:], in_=wproj_d[128 * kt : 128 * (kt + 1), :])
            wproj.append(t)
        # Q^T packed per head pair (rows 0-63 = head 2p, 64-127 = head 2p+1)
        qT = [sb.tile([128, L], B16, tag=f"qT{p}", bufs=1, name=f"qT{p}") for p in range(2)]
        # K^T zero-padded per head: kz[p][h] has head 2p+h in its own 64
        # rows, zeros elsewhere -> K=128 score matmuls pick out one head.
        kz = [
            [
                sb.tile([128, L], B16, tag=f"kz{p}{h}", bufs=1, name=f"kz{p}{h}")
                for h in range(2)
            ]
            for p in range(2)
        ]
        for p in range(2):
            nc.sync.dma_start(out=kz[p][0][64:128, :], in_=zer_d[:])
            nc.sync.dma_start(out=kz[p][1][0:64, :], in_=zer_d[:])
        # V natural layout, 16 token tiles of [128, 4*65]; col 64 of each
        # head group = 1.0 (from interleaved W zero-cols + bias ones row)
        vsb = [sb.tile([128, VW], B16, tag=f"v{t}", bufs=1, name=f"v{t}") for t in range(KT)]
        attnT = [sb.tile([128, L], R32, tag=f"attnT{k}", bufs=1, name=f"attnT{k}") for k in range(2)]

        # ================= QKV projection =================
        for s in range(NS):
            # Q/K part: out[wcol, token] = wqkv[:, m-tile].T @ xT
            for m in range(4):
                p_qk = ps.tile([128, 512], F32, tag="mm", bufs=2)
                for k in range(N_DK):
                    mm(
                        p_qk[:],
                        wqkv[k][:, 128 * m : 128 * (m + 1)],
                        xTc[k][s][:],
                        start=(k == 0),
                        stop=(k == N_DK - 1),
                    )
                # copy to SBUF (bf16) with per-partition (wcol) bias add
                cs = slice(512 * s, 512 * (s + 1))
                ID = mybir.ActivationFunctionType.Identity
                if m < 2:
                    nc.scalar.activation(qT[m][:, cs], p_qk[:], ID, bias=bqk[:, m : m + 1])
                else:
                    p = m - 2
                    nc.scalar.activation(
                        kz[p][0][0:64, cs], p_qk[0:64, :], ID, bias=bqk[0:64, m : m + 1]
                    )
                    nc.scalar.activation(
                        kz[p][1][64:128, cs], p_qk[64:128, :], ID, bias=bqk[64:128, m : m + 1]
                    )
            # V part: out[token, vcol] = xT[:, tt].T @ wv_interleaved
            for j in range(4):
                t = 4 * s + j
                p_v = ps.tile([128, VW], F32, tag="mm", bufs=2)
                for k in range(N_DK):
                    mm(
                        p_v[:],
                        xTc[k][s][:, 128 * j : 128 * (j + 1)],
                        wqkv[k][:, 2 * CD : 2 * CD + VW],
                        start=(k == 0),
                        stop=False,
                    )
                mm(p_v[:], ones[0:1, 0:128], bv[:], start=False, stop=True)
                nc.scalar.copy(vsb[t][:], p_v[:])

        # ================= attention =================
        # Software-pipelined across (pair, s) blocks:
        #  - AV matmuls issue AV_DELAY steps behind their exp (cross-block)
        #  - block b-1's tail AVs, 1/Z + unnormalized copy flush after block
        #    b's first steps; block b-2's normalize (bcast mm + mult) follows
        # so the PE never sits on an exp/reciprocal dependency.
        def emit_recip(av):
            # block end: pull Z and the unnormalized AV out of psum.  The
            # reciprocal happens NEXT block on the broadcast [64,512] tile --
            # a [1,512] reciprocal runs 512 elems on one DVE lane (~3.4us)
            # and blocks the mask adds queued behind it.
            rzs = []
            for h in range(2):
                z = sb.tile([1, 512], R32, tag="rz", bufs=4, name="z")
                nc.vector.tensor_copy(z[:], av[h][64:65, :])
                un = sb.tile([64, 512], F32, tag="un", bufs=4, name="un")
                nc.vector.tensor_copy(un[:], av[h][0:64, :])
                rzs.append((z, un))
            return rzs

        def emit_norm(pair, q0, rzs):
            for h in range(2):
                z, un = rzs[h]
                bc_ps = ps.tile([64, 512], F32, tag="mm", bufs=2, name="bc_ps")
                mm(bc_ps[:], ones[0:1, 0:64], z[:], start=True, stop=True)
                bc = sb.tile([64, 512], F32, tag="bc_sb", bufs=2, name="bc")
                with nc.allow_low_precision(reason="bcast 1/Z"):
                    nc.vector.reciprocal(bc[:], bc_ps[:])
                if h == 0:
                    nc.vector.tensor_tensor(
                        attnT[pair][0:64, q0 : q0 + 512],
                        un[:],
                        bc[:],
                        op=mybir.AluOpType.mult,
                    )
                else:
                    tmp = sb.tile([64, 512], R32, tag="ntmp", bufs=2, name="tmp")
                    nc.vector.tensor_tensor(
                        tmp[:], un[:], bc[:], op=mybir.AluOpType.mult
                    )
                    nc.sync.dma_start(
                        out=attnT[pair][64:128, q0 : q0 + 512], in_=tmp[:]
                    )

        pending = []  # (block_id, mm_args, mm_kwargs)
        fin_prev = None  # (block_id, pair, q0, av) awaiting tail-flush + recip
        norm_prev = None  # (pair, q0, rzs) awaiting normalize
        blocks = [(p, s) for p in range(2) for s in range(NS)]
        for bid, (pair, s) in enumerate(blocks):
            q0 = 512 * s
            n_k = 4 * s + 4
            av = [
                ps.tile([65, 512], F32, tag=f"av{h}", bufs=1, name=f"av{h}")
                for h in range(2)
            ]
            for k in range(n_k):
                k0 = 128 * k
                diag_t = k - 4 * s
                lo = 128 * diag_t if diag_t >= 0 else 0
                for h in range(2):
                    hg = 2 * pair + h
                    s_ps = ps.tile([128, 512], F32, tag="st", bufs=4)
                    mm(
                        s_ps[:, lo:512],
                        kz[pair][h][:, k0 : k0 + 128],
                        qT[pair][:, q0 + lo : q0 + 512],
                        start=True,
                        stop=True,
                    )
                    if diag_t >= 0:
                        nc.vector.tensor_tensor(
                            s_ps[:, lo : lo + 128],
                            s_ps[:, lo : lo + 128],
                            tri[:],
                            op=mybir.AluOpType.add,
                        )
                    pt = sb.tile([128, 512], B16, tag="pt", bufs=AV_DELAY + 2)
                    nc.scalar.activation(
                        pt[:, lo:512],
                        s_ps[:, lo:512],
                        mybir.ActivationFunctionType.Exp,
                        scale=SCALE,
                    )
                    pending.append(
                        (
                            bid,
                            (
                                av[h][0:65, lo:512],
                                vsb[k][:, 65 * hg : 65 * hg + 65],
                                pt[:, lo:512],
                            ),
                            dict(
                                start=(k == 0),
                                stop=(k == n_k - 1),
                                skip_group_check=True,
                            ),
                        )
                    )
                    while len(pending) > AV_DELAY:
                        _, a, kw = pending.pop(0)
                        mm(*a, **kw)
                if k == 1 and fin_prev is not None:
                    # flush the previous block's tail AVs, free its av psum
                    # via recip + unnormalized copy, then run the normalize
                    # of the block before that
                    pbid = fin_prev[0]
                    while pending and pending[0][0] == pbid:
                        _, a, kw = pending.pop(0)
                        mm(*a, **kw)
                    if norm_prev is not None:
                        emit_norm(*norm_prev)
                        norm_prev = None
                    _, ppair, pq0, pav = fin_prev
                    norm_prev = (ppair, pq0, emit_recip(pav))
                    fin_prev = None
            fin_prev = (bid, pair, q0, av)
        while pending:
            _, a, kw = pending.pop(0)
            mm(*a, **kw)
        if norm_prev is not None:
            emit_norm(*norm_prev)
        _, ppair, pq0, pav = fin_prev
        emit_norm(ppair, pq0, emit_recip(pav))

        # ================= output projection (partial) =================
        for s in range(NS):
            for m in range(N_DK):
                p_y = ps.tile([128, 512], F32, tag="mm", bufs=2)
                for kt in range(2):
                    mm(
                        p_y[:],
                        wproj[kt][:, 128 * m : 128 * (m + 1)],
                        attnT[kt][:, 512 * s : 512 * (s + 1)],
                        start=(kt == 0),
                        stop=(kt == 1),
                    )
                y_sb = sb.tile([128, 512], F32, tag="ysb", bufs=3)
                nc.scalar.activation(
                    y_sb[:], p_y[:], mybir.ActivationFunctionType.Identity,
                    bias=bproj[:, m : m + 1],
                )
                nc.sync.dma_start(
                    out=yT_d[128 * m : 128 * (m + 1), 512 * s : 512 * (s + 1)],
                    in_=y_sb[:],
                )
    _split_multi_waits(nc)
    return nc


_NC_CACHE = None
LAST_RESULTS = None

_ONESR = np.ones((1, 512), dtype=np.float32)
_ZER = np.zeros((64, L), dtype=NPB16)
_I, _J = np.meshgrid(np.arange(128), np.arange(128), indexing="ij")
_TRI = np.where(_J >= _I, 0.0, NEG).astype(np.float32)


def _make_in_maps(x, Wqkv, bqkv, Wproj, bproj):
    in_maps = []
    for c in range(N_CORES):
        b, g = divmod(c, 4)
        qc = slice(CD * g, CD * (g + 1))
        wq = Wqkv[:, qc]
        wk = Wqkv[:, D : 2 * D][:, qc]
        wv = Wqkv[:, 2 * D : 3 * D][:, qc]
        bq = bqkv[qc]
        bk = bqkv[D : 2 * D][qc]
        bvv = bqkv[2 * D : 3 * D][qc]
        # V columns interleaved per head: [wv_h (64 cols) | zeros col] so the
        # psum comes out in vsb layout; bv row gets [bv_h | 1.0].
        wv_i = np.zeros((D, VW), dtype=np.float32)
        bv_i = np.zeros((1, VW), dtype=np.float32)
        for h in range(HPC):
            wv_i[:, 65 * h : 65 * h + 64] = wv[:, 64 * h : 64 * h + 64]
            bv_i[0, 65 * h : 65 * h + 64] = bvv[64 * h : 64 * h + 64]
            bv_i[0, 65 * h + 64] = 1.0
        bqk_cols = np.concatenate([bq, bk]).reshape(4, 128).T  # [128, 4]
        in_maps.append(
            {
                "xT": np.ascontiguousarray(x[b].T),
                "wqkv": np.ascontiguousarray(
                    np.concatenate([wq, wk, wv_i], axis=1)
                ),
                "bqk": np.ascontiguousarray(bqk_cols),
                "bv": bv_i,
                "wproj": np.ascontiguousarray(Wproj[CD * g : CD * (g + 1), :]),
                "bproj": np.ascontiguousarray(
                    (bproj if g == 0 else np.zeros_like(bproj)).reshape(N_DK, 128).T
                ),
                "onesr": _ONESR,
                "trimask": _TRI,
                "zer": _ZER,
            }
        )

    return in_maps


def kernel(x, Wqkv, bqkv, Wproj, bproj):
    global _NC_CACHE, LAST_RESULTS
    x = np.asarray(x, dtype=np.float32)
    Wqkv = np.asarray(Wqkv, dtype=np.float32)
    bqkv = np.asarray(bqkv, dtype=np.float32)
    Wproj = np.asarray(Wproj, dtype=np.float32)
    bproj = np.asarray(bproj, dtype=np.float32)

    if _NC_CACHE is None:
        _NC_CACHE = _build_program()
    nc = _NC_CACHE

    in_maps = _make_in_maps(x, Wqkv, bqkv, Wproj, bproj)
    res = run_bass_kernel_spmd(nc, in_maps, core_ids=list(range(N_CORES)))
    LAST_RESULTS = res

    out = np.empty((B, L, D), dtype=np.float32)
    for b in range(B):
        acc = res.results[4 * b]["yT"].astype(np.float32)
        for g in range(1, 4):
            acc = acc + res.results[4 * b + g]["yT"]
        out[b] = acc.T
    return out



# revision 4
# speedup vs baseline: 1.0473x; 1.0473x over previous
"""Causal self-attention (B=2, L=2048, D=1024, H=16) on 8 trn2 NeuronCores.

Sharding: core c = 4*b + g handles batch b and head group g (4 heads).
Per core: QKV projection for its heads' weight columns (tensor-parallel),
flash-style causal attention for its 4 heads, and a partial output
projection over its 256 head-dims (row-parallel).  The host sums the 4
partial projections per batch and adds bproj.

Device layout: activations kept transposed (feature-major) throughout:
  xT [D, L] (f32r) -> Q^T packed per head-pair [128, L] (bf16);
  K^T stored as 4 zero-padded [128, L] bf16 tensors (head h occupies its
  64 partition rows, the other 64 rows are zero) so score matmuls run at
  full K=128 contraction with no tile_position tricks;
  V in natural [L, 4*65] bf16 layout -- col 64 of each head group is 1.0
  (produced by zero-cols in the interleaved weight + the bias ones-row
  matmul) so the attention row-sum Z rides along in the AV matmul;
  S^T tiles [k, q] so softmax needs no transposes; exp on ACT with the
  1/sqrt(hd) scale folded in; causal diagonal handled by adding a
  precomputed triangular -1e30 mask to the S^T psum.
QKV/proj matmuls run in float32r (rounded fp32); attention matmuls in
bf16 (separate overlapped weight loads, full PE rate).
"""

import sys
import types

import numpy as np


def _install_ntff_shim():
    """The container's antenv stub lacks axon_hooks; recreate it so
    run_bass_kernel_spmd(trace=True) can reach the NTFF profiler."""
    if "antenv.axon_hooks" in sys.modules:
        return
    try:
        import antenv
        from trn_agent_boot.trn_boot import _ntff_profile_via_ctypes
    except Exception:
        return
    mod = types.ModuleType("antenv.axon_hooks")
    hook = _ntff_profile_via_ctypes("/opt/axon/libaxon_pjrt.so")
    mod.get_axon_ntff_profile_hook = lambda: hook
    mod.set_axon_ntff_profile_hook = lambda h: None
    sys.modules["antenv.axon_hooks"] = mod
    antenv.axon_hooks = mod


_install_ntff_shim()

import ml_dtypes  # noqa: E402

import concourse.bass as bass  # noqa: E402
import concourse.mybir as mybir  # noqa: E402
import concourse.tile as tile  # noqa: E402
from concourse.bass_utils import run_bass_kernel_spmd  # noqa: E402
from concourse.vector_clock import ScopedClock, VectorClock  # noqa: E402

B, L, D, H = 2, 2048, 1024, 16
HD = D // H  # 64
N_CORES = 8
HPC = 4  # heads per core
CD = HPC * HD  # 256 head-dims per core
VW = HPC * (HD + 1)  # 260 interleaved V columns (64 vals + ones col per head)
SCALE = HD**-0.5  # 0.125
F32 = mybir.dt.float32
R32 = mybir.dt.float32r  # rounded-fp32 matmul format (4 bytes, np.float32)
B16 = mybir.dt.bfloat16
NPB16 = ml_dtypes.bfloat16
NEG = -1.0e30

KT = L // 128  # 16 k-tiles of 128 keys
NS = L // 512  # 4 query chunks of 512
N_DK = D // 128  # 8 feature k-tiles
AV_DELAY = 6  # AV matmul issues this many (k,h)-steps behind its exp


class _TileContext(tile.TileContext):
    """Split exit-drain sem waits to 1 per drain; this walrus build's
    CTRL codegen rejects drains with 2+ sync waits."""

    def _drain_and_barrier(self, tick_clock, wait_clock):
        g = tick_clock.global_clock
        n = len(g)
        procs = [i for i in range(n) if g[i] > 0]
        for p in procs:
            vec = [g[i] if i == p else 0 for i in range(n)]
            d = self.nc.sync.drain()
            wait_clock.add_sem_waits(d.ins, ScopedClock({None: VectorClock(vec)}))
        self.nc.all_engine_barrier()
        popped = self.nc._tile_sem_poison_stack.pop()
        assert popped is self._sem_poison
        self.nc.clear_and_free_semaphores(list(self.sems.allocated().values()))
        self.nc.all_engine_barrier()


def _split_multi_waits(nc):
    """This walrus build's codegen accepts only ONE sync wait per
    instruction; hoist extra waits onto preceding same-engine NOPs."""
    for f in nc.m.functions:
        for blk in f.blocks:
            orig = list(blk.instructions)
            expanded = []
            changed = False
            for ins in orig:
                si = ins.sync_info
                if si is not None and si.on_wait is not None and len(si.on_wait) > 1:
                    changed = True
                    waits = list(si.on_wait)
                    eng = nc.engines[ins.engine]
                    for w in waits[:-1]:
                        nop = eng.nop(nofuse=True).ins
                        # eng.nop() auto-appends to the CURRENT bb; pull it
                        # out -- we re-insert it before `ins` in ins's bb.
                        nc.cur_bb.bb.instructions.remove(nop)
                        nop.sync_info = mybir.SyncInfo(on_wait=[w], on_update=[])
                        expanded.append(nop)
                    ins.sync_info = mybir.SyncInfo(
                        on_wait=[waits[-1]], on_update=list(si.on_update or [])
                    )
                expanded.append(ins)
            if changed:
                il = blk.instructions
                for ins in list(il):
                    il.remove(ins)
                for ins in expanded:
                    il.append(ins)


def _build_program():
    nc = bass.Bass()
    xT_d = nc.dram_tensor("xT", [D, L], B16, kind="ExternalInput").ap()
    wqkv_d = nc.dram_tensor("wqkv", [D, 2 * CD + VW], B16, kind="ExternalInput").ap()
    bqk_d = nc.dram_tensor("bqk", [128, 4], F32, kind="ExternalInput").ap()
    bv_d = nc.dram_tensor("bv", [1, VW], B16, kind="ExternalInput").ap()
    wproj_d = nc.dram_tensor("wproj", [CD, D], B16, kind="ExternalInput").ap()
    bproj_d = nc.dram_tensor("bproj", [128, N_DK], F32, kind="ExternalInput").ap()
    onesr_d = nc.dram_tensor("onesr", [1, 512], B16, kind="ExternalInput").ap()
    tri_d = nc.dram_tensor("trimask", [128, 128], F32, kind="ExternalInput").ap()
    zer_d = nc.dram_tensor("zer", [64, L], B16, kind="ExternalInput").ap()
    yT_d = nc.dram_tensor("yT", [D, L], F32, kind="ExternalOutput").ap()

    mm = nc.tensor.matmul

    with _TileContext(nc) as tc, tc.tile_pool(name="sb", bufs=1) as sb, tc.tile_pool(
        name="ps", bufs=1, space="PSUM"
    ) as ps:
        # ---- constants (host-supplied; memset/affine_select of f32r
        # fail this walrus build's ISA checks) ----
        ones = sb.tile([1, 512], B16, tag="ones", bufs=1)
        nc.sync.dma_start(out=ones[:], in_=onesr_d[:])
        tri = sb.tile([128, 128], F32, tag="tri", bufs=1)
        nc.sync.dma_start(out=tri[:], in_=tri_d[:])
        bqk = sb.tile([128, 4], F32, tag="bqk", bufs=1)
        nc.sync.dma_start(out=bqk[:], in_=bqk_d[:])
        bv = sb.tile([1, VW], B16, tag="bv", bufs=1)
        nc.sync.dma_start(out=bv[:], in_=bv_d[:])
        bproj = sb.tile([128, N_DK], F32, tag="bproj", bufs=1)
        nc.sync.dma_start(out=bproj[:], in_=bproj_d[:])

        # ---- persistent SBUF tensors ----
        # xT loaded as [k, s] chunk tiles and wqkv split by column group so
        # the first QK accumulation only waits on ~2.5MB of DMA, not 11MB.
        wqkv = [
            sb.tile([128, 2 * CD + VW], B16, tag=f"wqkv{k}", bufs=1, name=f"wqkv{k}")
            for k in range(N_DK)
        ]
        wq_groups = [(0, 128), (128, 256), (256, 384), (384, 512), (512, 2 * CD + VW)]
        for k in range(N_DK):
            lo_, hi_ = wq_groups[0]
            nc.sync.dma_start(
                out=wqkv[k][:, lo_:hi_],
                in_=wqkv_d[128 * k : 128 * (k + 1), lo_:hi_],
            )
        xTc = [
            [
                sb.tile([128, 512], B16, tag=f"xT{k}_{s}", bufs=1, name=f"xT{k}_{s}")
                for s in range(NS)
            ]
            for k in range(N_DK)
        ]
        for k in range(N_DK):
            nc.sync.dma_start(
                out=xTc[k][0][:], in_=xT_d[128 * k : 128 * (k + 1), 0:512]
            )
        for lo_, hi_ in wq_groups[1:]:
            for k in range(N_DK):
                nc.sync.dma_start(
                    out=wqkv[k][:, lo_:hi_],
                    in_=wqkv_d[128 * k : 128 * (k + 1), lo_:hi_],
                )
        for s in range(1, NS):
            for k in range(N_DK):
                nc.sync.dma_start(
                    out=xTc[k][s][:],
                    in_=xT_d[128 * k : 128 * (k + 1), 512 * s : 512 * (s + 1)],
                )
        wproj = []
        for kt in range(2):
            t = sb.tile([128, D], B16, tag=f"wproj{kt}", bufs=1)
            nc.sync.dma_start(out=t[:], in_=wproj_d[128 * kt : 128 * (kt + 1), :])
            wproj.append(t)
        # Q^T packed per head pair (rows 0-63 = head 2p, 64-127 = head 2p+1)
        qT = [sb.tile([128, L], B16, tag=f"qT{p}", bufs=1, name=f"qT{p}") for p in range(2)]
        # K^T zero-padded per head: kz[p][h] has head 2p+h in its own 64
        # rows, zeros elsewhere -> K=128 score matmuls pick out one head.
        kz = [
            [
                sb.tile([128, L], B16, tag=f"kz{p}{h}", bufs=1, name=f"kz{p}{h}")
                for h in range(2)
            ]
            for p in range(2)
        ]
        for p in range(2):
            nc.sync.dma_start(out=kz[p][0][64:128, :], in_=zer_d[:])
            nc.sync.dma_start(out=kz[p][1][0:64, :], in_=zer_d[:])
        # V natural layout, 16 token tiles of [128, 4*65]; col 64 of each
        # head group = 1.0 (from interleaved W zero-cols + bias ones row)
        vsb = [sb.tile([128, VW], B16, tag=f"v{t}", bufs=1, name=f"v{t}") for t in range(KT)]
        attnT = [sb.tile([128, L], B16, tag=f"attnT{k}", bufs=1, name=f"attnT{k}") for k in range(2)]

        # ================= QKV projection =================
        for s in range(NS):
            # Q/K part: out[wcol, token] = wqkv[:, m-tile].T @ xT
            for m in range(4):
                p_qk = ps.tile([128, 512], F32, tag="mm", bufs=2)
                for k in range(N_DK):
                    mm(
                        p_qk[:],
                        wqkv[k][:, 128 * m : 128 * (m + 1)],
                        xTc[k][s][:],
                        start=(k == 0),
                        stop=(k == N_DK - 1),
                    )
                # copy to SBUF (bf16) with per-partition (wcol) bias add
                cs = slice(512 * s, 512 * (s + 1))
                ID = mybir.ActivationFunctionType.Identity
                if m < 2:
                    nc.scalar.activation(qT[m][:, cs], p_qk[:], ID, bias=bqk[:, m : m + 1])
                else:
                    p = m - 2
                    nc.scalar.activation(
                        kz[p][0][0:64, cs], p_qk[0:64, :], ID, bias=bqk[0:64, m : m + 1]
                    )
                    nc.scalar.activation(
                        kz[p][1][64:128, cs], p_qk[64:128, :], ID, bias=bqk[64:128, m : m + 1]
                    )
            # V part: out[token, vcol] = xT[:, tt].T @ wv_interleaved
            for j in range(4):
                t = 4 * s + j
                p_v = ps.tile([128, VW], F32, tag="mm", bufs=2)
                for k in range(N_DK):
                    mm(
                        p_v[:],
                        xTc[k][s][:, 128 * j : 128 * (j + 1)],
                        wqkv[k][:, 2 * CD : 2 * CD + VW],
                        start=(k == 0),
                        stop=False,
                    )
                mm(p_v[:], ones[0:1, 0:128], bv[:], start=False, stop=True)
                nc.scalar.copy(vsb[t][:], p_v[:])

        # ================= attention =================
        # Software-pipelined across (pair, s) blocks:
        #  - AV matmuls issue AV_DELAY steps behind their exp (cross-block)
        #  - block b-1's tail AVs, 1/Z + unnormalized copy flush after block
        #    b's first steps; block b-2's normalize (bcast mm + mult) follows
        # so the PE never sits on an exp/reciprocal dependency.
        def emit_recip(av):
            # block end: pull Z and the unnormalized AV out of psum.  The
            # reciprocal happens NEXT block on the broadcast [64,512] tile --
            # a [1,512] reciprocal runs 512 elems on one DVE lane (~3.4us)
            # and blocks the mask adds queued behind it.
            rzs = []
            for h in range(2):
                z = sb.tile([1, 512], B16, tag="rz", bufs=4, name="z")
                nc.vector.tensor_copy(z[:], av[h][64:65, :])
                un = sb.tile([64, 512], F32, tag="un", bufs=4, name="un")
                nc.vector.tensor_copy(un[:], av[h][0:64, :])
                rzs.append((z, un))
            return rzs

        def emit_norm(pair, q0, rzs):
            for h in range(2):
                z, un = rzs[h]
                bc_ps = ps.tile([64, 512], F32, tag="mm", bufs=2, name="bc_ps")
                mm(bc_ps[:], ones[0:1, 0:64], z[:], start=True, stop=True)
                bc = sb.tile([64, 512], F32, tag="bc_sb", bufs=2, name="bc")
                with nc.allow_low_precision(reason="bcast 1/Z"):
                    nc.vector.reciprocal(bc[:], bc_ps[:])
                if h == 0:
                    nc.vector.tensor_tensor(
                        attnT[pair][0:64, q0 : q0 + 512],
                        un[:],
                        bc[:],
                        op=mybir.AluOpType.mult,
                    )
                else:
                    tmp = sb.tile([64, 512], B16, tag="ntmp", bufs=2, name="tmp")
                    nc.vector.tensor_tensor(
                        tmp[:], un[:], bc[:], op=mybir.AluOpType.mult
                    )
                    nc.sync.dma_start(
                        out=attnT[pair][64:128, q0 : q0 + 512], in_=tmp[:]
                    )

        pending = []  # (block_id, mm_args, mm_kwargs)
        fin_prev = None  # (block_id, pair, q0, av) awaiting tail-flush + recip
        norm_prev = None  # (pair, q0, rzs) awaiting normalize
        blocks = [(p, s) for p in range(2) for s in range(NS)]
        for bid, (pair, s) in enumerate(blocks):
            q0 = 512 * s
            n_k = 4 * s + 4
            av = [
                ps.tile([65, 512], F32, tag=f"av{h}", bufs=1, name=f"av{h}")
                for h in range(2)
            ]
            for k in range(n_k):
                k0 = 128 * k
                diag_t = k - 4 * s
                lo = 128 * diag_t if diag_t >= 0 else 0
                for h in range(2):
                    hg = 2 * pair + h
                    s_ps = ps.tile([128, 512], F32, tag="st", bufs=4)
                    mm(
                        s_ps[:, lo:512],
                        kz[pair][h][:, k0 : k0 + 128],
                        qT[pair][:, q0 + lo : q0 + 512],
                        start=True,
                        stop=True,
                    )
                    if diag_t >= 0:
                        nc.vector.tensor_tensor(
                            s_ps[:, lo : lo + 128],
                            s_ps[:, lo : lo + 128],
                            tri[:],
                            op=mybir.AluOpType.add,
                        )
                    pt = sb.tile([128, 512], B16, tag="pt", bufs=AV_DELAY + 2)
                    nc.scalar.activation(
                        pt[:, lo:512],
                        s_ps[:, lo:512],
                        mybir.ActivationFunctionType.Exp,
                        scale=SCALE,
                    )
                    pending.append(
                        (
                            bid,
                            (
                                av[h][0:65, lo:512],
                                vsb[k][:, 65 * hg : 65 * hg + 65],
                                pt[:, lo:512],
                            ),
                            dict(
                                start=(k == 0),
                                stop=(k == n_k - 1),
                                skip_group_check=True,
                            ),
                        )
                    )
                    while len(pending) > AV_DELAY:
                        _, a, kw = pending.pop(0)
                        mm(*a, **kw)
                if k == 1 and fin_prev is not None:
                    # flush the previous block's tail AVs, free its av psum
                    # via recip + unnormalized copy, then run the normalize
                    # of the block before that
                    pbid = fin_prev[0]
                    while pending and pending[0][0] == pbid:
                        _, a, kw = pending.pop(0)
                        mm(*a, **kw)
                    if norm_prev is not None:
                        emit_norm(*norm_prev)
                        norm_prev = None
                    _, ppair, pq0, pav = fin_prev
                    norm_prev = (ppair, pq0, emit_recip(pav))
                    fin_prev = None
            fin_prev = (bid, pair, q0, av)
        while pending:
            _, a, kw = pending.pop(0)
            mm(*a, **kw)
        if norm_prev is not None:
            emit_norm(*norm_prev)
        _, ppair, pq0, pav = fin_prev
        emit_norm(ppair, pq0, emit_recip(pav))

        # ================= output projection (partial) =================
        for s in range(NS):
            for m in range(N_DK):
                p_y = ps.tile([128, 512], F32, tag="mm", bufs=2)
                for kt in range(2):
                    mm(
                        p_y[:],
                        wproj[kt][:, 128 * m : 128 * (m + 1)],
                        attnT[kt][:, 512 * s : 512 * (s + 1)],
                        start=(kt == 0),
                        stop=(kt == 1),
                    )
                y_sb = sb.tile([128, 512], F32, tag="ysb", bufs=3)
                nc.scalar.activation(
                    y_sb[:], p_y[:], mybir.ActivationFunctionType.Identity,
                    bias=bproj[:, m : m + 1],
                )
                nc.sync.dma_start(
                    out=yT_d[128 * m : 128 * (m + 1), 512 * s : 512 * (s + 1)],
                    in_=y_sb[:],
                )
    _split_multi_waits(nc)
    return nc


_NC_CACHE = None
LAST_RESULTS = None

_ONESR = np.ones((1, 512), dtype=NPB16)
_ZER = np.zeros((64, L), dtype=NPB16)
_I, _J = np.meshgrid(np.arange(128), np.arange(128), indexing="ij")
_TRI = np.where(_J >= _I, 0.0, NEG).astype(np.float32)


def _make_in_maps(x, Wqkv, bqkv, Wproj, bproj):
    in_maps = []
    for c in range(N_CORES):
        b, g = divmod(c, 4)
        qc = slice(CD * g, CD * (g + 1))
        wq = Wqkv[:, qc]
        wk = Wqkv[:, D : 2 * D][:, qc]
        wv = Wqkv[:, 2 * D : 3 * D][:, qc]
        bq = bqkv[qc]
        bk = bqkv[D : 2 * D][qc]
        bvv = bqkv[2 * D : 3 * D][qc]
        # V columns interleaved per head: [wv_h (64 cols) | zeros col] so the
        # psum comes out in vsb layout; bv row gets [bv_h | 1.0].
        wv_i = np.zeros((D, VW), dtype=np.float32)
        bv_i = np.zeros((1, VW), dtype=np.float32)
        for h in range(HPC):
            wv_i[:, 65 * h : 65 * h + 64] = wv[:, 64 * h : 64 * h + 64]
            bv_i[0, 65 * h : 65 * h + 64] = bvv[64 * h : 64 * h + 64]
            bv_i[0, 65 * h + 64] = 1.0
        bqk_cols = np.concatenate([bq, bk]).reshape(4, 128).T  # [128, 4]
        in_maps.append(
            {
                "xT": np.ascontiguousarray(x[b].T.astype(NPB16)),
                "wqkv": np.ascontiguousarray(
                    np.concatenate([wq, wk, wv_i], axis=1).astype(NPB16)
                ),
                "bqk": np.ascontiguousarray(bqk_cols),
                "bv": bv_i.astype(NPB16),
                "wproj": np.ascontiguousarray(Wproj[CD * g : CD * (g + 1), :].astype(NPB16)),
                "bproj": np.ascontiguousarray(
                    (bproj if g == 0 else np.zeros_like(bproj)).reshape(N_DK, 128).T
                ),
                "onesr": _ONESR,
                "trimask": _TRI,
                "zer": _ZER,
            }
        )

    return in_maps


def kernel(x, Wqkv, bqkv, Wproj, bproj):
    global _NC_CACHE, LAST_RESULTS
    x = np.asarray(x, dtype=np.float32)
    Wqkv = np.asarray(Wqkv, dtype=np.float32)
    bqkv = np.asarray(bqkv, dtype=np.float32)
    Wproj = np.asarray(Wproj, dtype=np.float32)
    bproj = np.asarray(bproj, dtype=np.float32)

    if _NC_CACHE is None:
        _NC_CACHE = _build_program()
    nc = _NC_CACHE

    in_maps = _make_in_maps(x, Wqkv, bqkv, Wproj, bproj)
    res = run_bass_kernel_spmd(nc, in_maps, core_ids=list(range(N_CORES)))
    LAST_RESULTS = res

    out = np.empty((B, L, D), dtype=np.float32)
    for b in range(B):
        acc = res.results[4 * b]["yT"].astype(np.float32)
        for g in range(1, 4):
            acc = acc + res.results[4 * b + g]["yT"]
        out[b] = acc.T
    return out

